# revision 37
# baseline (speedup 1.0000x reference)
"""Compressed-KV GPT-2 attention block on 8 TRN2 NeuronCores.

Sharding: batch x head-group. Core c: batch b = c//4, heads 4*(c%4)..+4.

The KV compressor is linear + low-rank, so everything folds on host:
  scores = q @ (k_c wk_d)^T / 8 = (q wk_d^T / 8) @ k_c^T   -> q' [S,32]
  v_c    = v @ wv_c = hidden @ (w_v wv_c) + b_v wv_c       -> direct projection
  out_h  = (P @ v_c) @ (wv_d w_proj_h)                     -> attn in C=32 space
so the device pipeline works entirely in the compressed C=32 head space:
  qkv'  : hidden^T -> q'^T, k_c^T, v_c^T (32 rows/head, 3 m-blocks)
  vpack : per key tile, PE-transpose v_c^T [128,128] -> v_c [keys, 4hx32c]
  S^T   : k_c^T slices^T @ q'^T, 2+2 heads packed via 32x128-mode PE row
          tiling into two 2-bank PSUM tiles (scA/scB); exp on ScalarE per half
          ([128,~1024] strided) with zero ping-pong bubbles
  attn  : v_c^T @ E + ones^T @ E (denominator), 4 heads packed via 128x32-mode
          PE column tiling; normalize by DMA-bounced reciprocal broadcast
  out^T : w_proj'^T-chunks @ attn_norm (K=128 covers all 4 heads at C=32)
Host sums the 4 partials per batch and adds b_proj.

The PE instruction stream hand-interleaves qkv/vpack/proj filler work into the
score/attnV stream so the PE stays dense (HAM-warm) while pacing the ScalarE
exp stream (the bottleneck). The last q-block's normalize + c_proj are split
into column halves so most of the tail overlaps the final exps.
"""

import sys

if "/opt/trn_rl_repo" not in sys.path:
    sys.path.insert(0, "/opt/trn_rl_repo")

import numpy as np
import ml_dtypes

BF16 = ml_dtypes.bfloat16

B, S, D = 2, 2048, 1024
H, hd, C = 16, 64, 32
NCORES = 8
HPC = 4            # heads per core
SB = 512           # q block
NSB = S // SB      # 4
KT = 128           # keys per tile
NKT = S // KT      # 16
DC = D // 128      # 8 contraction chunks
MB = 3             # qkv m-blocks: q', k_c, v_c

_cache = {}


def _build():
    import concourse.bacc as bacc
    import concourse.tile as tile
    import concourse.mybir as mybir

    dt = mybir.dt
    f32, bf16 = dt.float32, dt.bfloat16
    Exp = mybir.ActivationFunctionType.Exp
    mult = mybir.AluOpType.mult

    nc = bacc.Bacc("TRN2", target_bir_lowering=False, debug=False, num_devices=NCORES)

    hidden_t = nc.dram_tensor("hidden_t", [D, S], bf16, kind="ExternalInput")
    w_qkv = nc.dram_tensor("w_qkv", [D, MB * 128], bf16, kind="ExternalInput")
    b_qkv = nc.dram_tensor("b_qkv", [128, MB], f32, kind="ExternalInput")
    w_projp = nc.dram_tensor("w_projp", [128, D], bf16, kind="ExternalInput")
    mask_in = nc.dram_tensor("mask_in", [128, 4 * KT], bf16, kind="ExternalInput")
    ident_in = nc.dram_tensor("ident_in", [128, 128], bf16, kind="ExternalInput")
    out_t = nc.dram_tensor("out_t", [D, S], bf16, kind="ExternalOutput")

    with tile.TileContext(nc) as tc:
        with (
            tc.tile_pool(name="persist", bufs=1) as pp,
            tc.tile_pool(name="epool", bufs=6) as ep,
            tc.tile_pool(name="npool", bufs=2) as npo,
            tc.tile_pool(name="ostage", bufs=6) as op,
            tc.tile_pool(name="dscr", bufs=2, space="DRAM") as dr,
            tc.tile_pool(name="ps_scA", bufs=1, space="PSUM") as ps_scA,
            tc.tile_pool(name="ps_scB", bufs=1, space="PSUM") as ps_scB,
            tc.tile_pool(name="ps_at", bufs=1, space="PSUM") as ps_at,
            tc.tile_pool(name="ps_dn", bufs=1, space="PSUM") as ps_dn,
            tc.tile_pool(name="ps_big", bufs=2, space="PSUM") as ps_big,
        ):
            # ---- weights + hidden, few big DMAs, in consumption order ----
            wq_all = pp.tile([128, DC * MB * 128], bf16, tag="wq", name="wq_all")
            wqv = wq_all[:].rearrange("p (d c) -> p d c", d=DC)
            nc.sync.dma_start(
                wqv[:, :, 0:128],
                w_qkv.ap()[:, 0:128].rearrange("(d p) c -> p d c", d=DC),
            )
            bias = pp.tile([128, MB], f32, tag="bias", name="bias")
            nc.sync.dma_start(bias[:], b_qkv.ap())
            hT_all = pp.tile([128, DC * S], bf16, tag="hT", name="hT_all")
            hv = hT_all[:].rearrange("p (d s) -> p d s", d=DC)
            for q in range(2):
                nc.scalar.dma_start(
                    hv[:, 4 * q:4 * q + 4, 0:SB],
                    hidden_t.ap()[512 * q:512 * q + 512, 0:SB]
                    .rearrange("(d p) s -> p d s", d=4),
                )
            nc.sync.dma_start(
                wqv[:, :, 128:256],
                w_qkv.ap()[:, 128:256].rearrange("(d p) c -> p d c", d=DC),
            )
            nc.sync.dma_start(
                wqv[:, :, 256:384],
                w_qkv.ap()[:, 256:384].rearrange("(d p) c -> p d c", d=DC),
            )
            ident = pp.tile([128, 128], bf16, tag="ident", name="ident")
            nc.sync.dma_start(ident[:], ident_in.ap())
            mask4 = pp.tile([128, 4 * KT], bf16, tag="mask4", name="mask4")
            nc.sync.dma_start(mask4[:], mask_in.ap())
            nc.scalar.dma_start(
                hv[:, :, SB:S],
                hidden_t.ap()[:, SB:S].rearrange("(d p) s -> p d s", d=DC),
            )
            wpj = pp.tile([128, D], bf16, tag="wpj", name="wpj")
            nc.sync.dma_start(wpj[:], w_projp.ap())

            def wq_sl(d, mb):
                return wq_all[:, d * (MB * 128) + mb * 128:d * (MB * 128) + (mb + 1) * 128]

            def hT_sl(d, lo, hi):
                return hT_all[:, d * S + lo:d * S + hi]

            ones = pp.tile([128, 1], bf16, tag="ones", name="ones")
            nc.vector.memset(ones[:], 1.0)
            # preload the Exp table set early (off the critical path)
            warm = pp.tile([128, 1], bf16, tag="warm", name="warm")
            nc.scalar.activation(warm[:], ones[:], Exp)

            # qkv'^T destinations
            qp = pp.tile([128, S], bf16, tag="qp", name="qp")
            kcT = pp.tile([128, S], bf16, tag="kcT", name="kcT")
            vcT = pp.tile([128, S], bf16, tag="vcT", name="vcT")
            dests = [qp, kcT, vcT]
            vpack = [pp.tile([128, 128], bf16, tag=f"vpk{t}", name=f"vpk{t}")
                     for t in range(NKT)]

            def qkv_group(sb, mb, lo=0, hi=SB):
                ps = ps_big.tile([128, SB], f32, tag="big", name="psQ")
                w = hi - lo
                for d in range(DC):
                    nc.tensor.matmul(
                        ps[:, 0:w],
                        wq_sl(d, mb),
                        hT_sl(d, sb * SB + lo, sb * SB + hi),
                        start=(d == 0),
                        stop=(d == DC - 1),
                    )
                nc.vector.tensor_scalar_add(
                    out=dests[mb][:, sb * SB + lo:sb * SB + hi],
                    in0=ps[:, 0:w],
                    scalar1=bias[:, mb:mb + 1],
                )

            def vc_tr(kt):
                # vpack[kt][key, 32h+c] = vcT[32h+c, kt*128+key] via PE transpose
                psT = ps_big.tile([128, 128], bf16, tag="big", name="psT")
                nc.tensor.transpose(
                    psT[:], vcT[:, kt * KT:(kt + 1) * KT], ident[:]
                )
                nc.vector.tensor_copy(vpack[kt][:], psT[:])

            es = {}

            def scores_half(qsb, kb, half):
                # half 0: heads 0,1 -> scA; half 1: heads 2,3 -> scB
                r = kb - 4 * qsb
                c0 = max(r, 0) * KT
                pool = ps_scA if half == 0 else ps_scB
                sc = pool.tile([128, 2 * SB], f32, tag=f"sc{half}", name="sc")
                for hh in range(2):
                    h = 2 * half + hh
                    nc.tensor.matmul(
                        sc[:, hh * SB + c0:(hh + 1) * SB],
                        kcT[32 * h:32 * h + 32, kb * KT:(kb + 1) * KT],
                        qp[32 * h:32 * h + 32, qsb * SB + c0:(qsb + 1) * SB],
                        tile_position=(32 * h, 0),
                    )
                if half == 0:
                    es[(qsb, kb)] = ep.tile([128, 4 * SB], bf16, tag="e", name="e")
                e = es[(qsb, kb)]
                scv = sc[:].rearrange("p (h w) -> p h w", h=2)[:, :, c0:SB]
                ev = e[:].rearrange("p (h w) -> p h w", h=4)[:, 2 * half:2 * half + 2, c0:SB]
                nc.scalar.activation(ev, scv, Exp)
                if r >= 0 and half == 1:
                    # mask the diagonal 128-col block of each head (all 4)
                    ed = e[:].rearrange("p (h w) -> p h w", h=4)[:, :, c0:c0 + KT]
                    mv = mask4[:].rearrange("p (h w) -> p h w", h=4)
                    nc.vector.tensor_tensor(ed, ed, mv, mult)

            def scores_group(qsb, kb):
                scores_half(qsb, kb, 0)
                scores_half(qsb, kb, 1)

            attn_ps = {}
            den_ps = {}

            def attn_group(qsb, kb):
                r = kb - 4 * qsb
                c0 = max(r, 0) * KT
                nkb = 4 * qsb + 4
                if kb == 0:
                    attn_ps[qsb] = ps_at.tile([128, SB], f32, tag="at", name="at")
                    den_ps[qsb] = ps_dn.tile([128, SB], f32, tag="dn", name="dn")
                at, dn = attn_ps[qsb], den_ps[qsb]
                e = es.pop((qsb, kb))
                for h in range(HPC):
                    ee = e[:, h * SB + c0:(h + 1) * SB]
                    nc.tensor.matmul(
                        at[32 * h:32 * h + 32, c0:SB], vpack[kb][:, 32 * h:32 * h + 32],
                        ee, tile_position=(0, 32 * h),
                        start=(kb == 0), stop=(kb == nkb - 1),
                    )
                    nc.tensor.matmul(
                        dn[32 * h:32 * h + 1, c0:SB], ones[:, 0:1],
                        ee, tile_position=(0, 32 * h),
                        start=(kb == 0), stop=(kb == nkb - 1),
                    )

            attn_norm = {}

            def norm(qsb, lo, hi, key, eng=None):
                # normalize attn columns [lo, hi) of q-block qsb
                w = hi - lo
                eng = eng or nc.sync
                at, dn = attn_ps[qsb], den_ps[qsb]
                attn_sb = npo.tile([128, w], bf16, tag=f"attn_sb{w}", name="attn_sb")
                nc.vector.tensor_copy(attn_sb[:], at[:, lo:hi])
                den_sb = npo.tile([128, w], bf16, tag=f"den_sb{w}", name="den_sb")
                nc.vector.tensor_copy(den_sb[:], dn[:, lo:hi])
                # bounce den rows {32h} -> [128, w/32] for a cheap DVE reciprocal
                den_dr = dr.tile([HPC, w], bf16, tag=f"den_dr{w}", name="den_dr")
                eng.dma_start(
                    den_dr[:], den_sb[:].rearrange("(h s) q -> h s q", s=32)[:, 0, :]
                )
                j = w // 32
                den_c = npo.tile([128, j], bf16, tag=f"den_c{w}", name="den_c")
                eng.dma_start(
                    den_c[:], den_dr[:].rearrange("h (p j) -> (h p) j", j=j)
                )
                rec_c = npo.tile([128, j], bf16, tag=f"rec_c{w}", name="rec_c")
                with nc.allow_low_precision(reason="softmax denom recip in bf16"):
                    nc.vector.reciprocal(rec_c[:], den_c[:])
                rec_dr = dr.tile([HPC, w], bf16, tag=f"rec_dr{w}", name="rec_dr")
                eng.dma_start(
                    rec_dr[:].rearrange("h (p j) -> (h p) j", j=j), rec_c[:]
                )
                recb = npo.tile([128, w], bf16, tag=f"recb{w}", name="recb")
                for h in range(HPC):
                    eng.dma_start(
                        recb[32 * h:32 * h + 32, :],
                        rec_dr[h:h + 1, :].to_broadcast([32, w]),
                    )
                an = npo.tile([128, w], bf16, tag=f"an{w}", name="an")
                nc.vector.tensor_tensor(an[:], attn_sb[:], recb[:], mult)
                attn_norm[key] = (an, lo, hi)

            def proj_group(key, mb, tail=False):
                an, lo, hi = attn_norm[key]
                qsb = key if isinstance(key, int) else key[0]
                w = hi - lo
                ps = ps_big.tile([128, SB], f32, tag="big", name="psP")
                nc.tensor.matmul(ps[:, 0:w], wpj[:, mb * 128:(mb + 1) * 128], an[:])
                stage = op.tile([128, w], bf16, tag=f"stage{w}", name="stage")
                # in the tail the exps are done: route evac+DMA through the
                # idle Scalar engine for every other block to halve the chain
                if tail and mb % 2 == 1:
                    nc.scalar.copy(stage[:], ps[:, 0:w])
                    dma_eng = nc.scalar
                else:
                    nc.vector.tensor_copy(stage[:], ps[:, 0:w])
                    dma_eng = nc.sync
                dma_eng.dma_start(
                    out_t.ap()[mb * 128:(mb + 1) * 128,
                               qsb * SB + lo:qsb * SB + hi],
                    stage[:],
                )

            # ================= PE schedule =================
            def QKV(sb, mb):
                return lambda: qkv_group(sb, mb)

            def VCT(kt):
                return lambda: vc_tr(kt)

            def PRJ(key, mb):
                return lambda: proj_group(key, mb)

            def NRM3A():
                def f():
                    norm(3, 0, 256, (3, 0))
                    for mb in range(8):
                        proj_group((3, 0), mb, tail=True)
                return f

            def NRM(qsb):
                def f():
                    norm(qsb, 0, SB, qsb)
                    attn_ps.pop(qsb)
                    den_ps.pop(qsb)
                return f

            # flat stream of all 40 (qsb, kb) score groups; the attnV stream
            # lags one group behind globally (including across qsb boundaries,
            # so the exp stream never stalls at a boundary); fillers per slot
            groups = [(q, k) for q in range(NSB) for k in range(4 * q + 4)]
            fillers = {
                1: [QKV(1, 0)], 2: [VCT(2)], 3: [VCT(3)],
                5: [QKV(1, 1)], 6: [QKV(1, 2)], 7: [QKV(2, 0)],
                8: [VCT(4)], 9: [VCT(5)], 10: [VCT(6)], 11: [VCT(7)],
                13: [QKV(2, 1)], 14: [QKV(2, 2)], 15: [QKV(3, 0)],
                16: [PRJ(0, 0)], 17: [PRJ(0, 1), VCT(8)],
                18: [PRJ(0, 2), VCT(9)], 19: [PRJ(0, 3), VCT(10)],
                20: [PRJ(0, 4), VCT(11)], 21: [PRJ(0, 5)], 22: [PRJ(0, 6)],
                23: [PRJ(0, 7)],
                25: [QKV(3, 1)], 26: [QKV(3, 2)], 27: [PRJ(1, 0)],
                28: [PRJ(1, 1)], 29: [PRJ(1, 2)], 30: [PRJ(1, 3)],
                31: [PRJ(1, 4)], 32: [PRJ(1, 5)],
                33: [PRJ(1, 6), VCT(12)], 34: [PRJ(1, 7), VCT(13)],
                35: [PRJ(2, 0), VCT(14)], 36: [PRJ(2, 1), VCT(15)],
                37: [PRJ(2, 2), PRJ(2, 3)], 38: [PRJ(2, 4), PRJ(2, 5)],
                39: [PRJ(2, 6), PRJ(2, 7)],
            }

            # lead-in: q' of sb0, then k_c keys 0:256 only, so the first
            # scores+exp fire before the rest of sb0's projections
            qkv_group(0, 0)
            qkv_group(0, 1, 0, 256)
            scores_group(0, 0)
            scores_group(0, 1)
            qkv_group(0, 1, 256, SB)
            qkv_group(0, 2)
            vc_tr(0)
            vc_tr(1)

            for i, (qsb, kb) in enumerate(groups):
                if i == 38:
                    # qsb3 col 0:256 of attn/den is final after A(3,13):
                    # kick its normalize early so proj3a overlaps the tail
                    attn_group(3, 13)
                    norm(3, 0, 256, (3, 0))
                if i > 1:
                    scores_group(qsb, kb)
                pq, pk = groups[i - 1] if i > 0 else (None, None)
                if i > 0 and not (pq == 3 and pk == 13):
                    attn_group(pq, pk)
                    if pq < qsb:
                        norm(pq, 0, SB, pq)
                        attn_ps.pop(pq)
                        den_ps.pop(pq)
                for thunk in fillers.get(i, []):
                    thunk()
            attn_group(3, 15)
            # tail: second half of qsb3 (norm3b DMAs ride the idle ACT queue)
            norm(3, 256, SB, (3, 1), eng=nc.scalar)
            attn_ps.pop(3)
            den_ps.pop(3)
            for mb in range(8):
                proj_group((3, 0), mb, tail=True)
            for mb in range(8):
                proj_group((3, 1), mb, tail=True)

    nc.compile()
    return nc


def _prep_inputs(hidden_states, w_attn, b_attn, wk_c, wv_c, wk_d, wv_d, w_proj):
    """Per-core input maps (host-side shard + fold + pack + bf16 cast)."""
    f8 = np.float64
    hidden_T = [np.ascontiguousarray(hidden_states[b].T).astype(BF16) for b in range(B)]
    w_attn8, b_attn8 = w_attn.astype(f8), b_attn.astype(f8)
    scl = 1.0 / np.sqrt(hd)

    k = np.arange(128).reshape(128, 1)
    j = np.arange(KT).reshape(1, KT)
    mask1 = (k <= j).astype(BF16)
    mask4 = np.tile(mask1, (1, 4))
    ident = np.eye(128, dtype=BF16)

    in_maps = []
    for c in range(NCORES):
        b = c // 4
        hs = [4 * (c % 4) + h for h in range(HPC)]
        wq_cols, bq = [], []
        wk_cols, bk = [], []
        wv_cols, bv = [], []
        for h in hs:
            wq_cols.append(w_attn8[:, h * hd:(h + 1) * hd] @ wk_d[h].astype(f8).T * scl)
            bq.append(b_attn8[h * hd:(h + 1) * hd] @ wk_d[h].astype(f8).T * scl)
            wk_cols.append(w_attn8[:, D + h * hd:D + (h + 1) * hd] @ wk_c[h].astype(f8))
            bk.append(b_attn8[D + h * hd:D + (h + 1) * hd] @ wk_c[h].astype(f8))
            wv_cols.append(
                w_attn8[:, 2 * D + h * hd:2 * D + (h + 1) * hd] @ wv_c[h].astype(f8))
            bv.append(b_attn8[2 * D + h * hd:2 * D + (h + 1) * hd] @ wv_c[h].astype(f8))
        w_qkv_l = np.concatenate(
            [np.concatenate(wq_cols, 1), np.concatenate(wk_cols, 1),
             np.concatenate(wv_cols, 1)], axis=1,
        ).astype(BF16)                                          # [1024, 384]
        b_qkv_l = np.stack(
            [np.concatenate(bq), np.concatenate(bk), np.concatenate(bv)],
            axis=1,
        ).astype(np.float32)                                    # [128, 3]
        w_projp_l = np.concatenate(
            [wv_d[h].astype(f8) @ w_proj[h * hd:(h + 1) * hd, :].astype(f8)
             for h in hs], axis=0,
        ).astype(BF16)                                          # [128, 1024]
        in_maps.append(
            {
                "hidden_t": hidden_T[b],
                "w_qkv": w_qkv_l,
                "b_qkv": b_qkv_l,
                "w_projp": w_projp_l,
                "mask_in": mask4,
                "ident_in": ident,
            }
        )
    return in_maps


def kernel(
    hidden_states,
    w_attn,
    b_attn,
    w_proj,
    b_proj,
    wk_c,
    wv_c,
    wk_d,
    wv_d,
    _trace=False,
):
    from concourse.bass_utils import run_bass_kernel_spmd

    if "nc" not in _cache:
        _cache["nc"] = _build()
    nc = _cache["nc"]

    in_maps = _prep_inputs(
        np.asarray(hidden_states),
        np.asarray(w_attn),
        np.asarray(b_attn),
        np.asarray(wk_c),
        np.asarray(wv_c),
        np.asarray(wk_d),
        np.asarray(wv_d),
        np.asarray(w_proj),
    )
    res = run_bass_kernel_spmd(
        nc, in_maps, core_ids=list(range(NCORES)), trace=_trace
    )
    out = np.empty((B, S, D), np.float32)
    for b in range(B):
        acc = np.zeros((D, S), np.float32)
        for c in range(4 * b, 4 * b + 4):
            acc += res.results[c]["out_t"].astype(np.float32)
        out[b] = acc.T + np.asarray(b_proj, np.float32)
    if _trace:
        _cache["last_exec_time_ns"] = res.exec_time_ns
        _cache["last_results"] = res
    return out


# revision 38
# speedup vs baseline: 1.0339x; 1.0339x over previous
"""Compressed-KV GPT-2 attention block on 8 TRN2 NeuronCores.

Sharding: batch x head-group. Core c: batch b = c//4, heads 4*(c%4)..+4.

The KV compressor is linear + low-rank, so everything folds on host:
  scores = q @ (k_c wk_d)^T / 8 = (q wk_d^T / 8) @ k_c^T   -> q' [S,32]
  v_c    = v @ wv_c = hidden @ (w_v wv_c) + b_v wv_c       -> direct projection
  out_h  = (P @ v_c) @ (wv_d w_proj_h)                     -> attn in C=32 space
so the device pipeline works entirely in the compressed C=32 head space:
  qkv'  : hidden^T -> q'^T, k_c^T, v_c^T (32 rows/head, 3 m-blocks)
  vpack : per key tile, PE-transpose v_c^T [128,128] -> v_c [keys, 4hx32c]
  S^T   : k_c^T slices^T @ q'^T, 2+2 heads packed via 32x128-mode PE row
          tiling into two 2-bank PSUM tiles (scA/scB); exp on ScalarE per half
          ([128,~1024] strided) with zero ping-pong bubbles
  attn  : v_c^T @ E + ones^T @ E (denominator), 4 heads packed via 128x32-mode
          PE column tiling; normalize by DMA-bounced reciprocal broadcast
  out^T : w_proj'^T-chunks @ attn_norm (K=128 covers all 4 heads at C=32)
Host sums the 4 partials per batch and adds b_proj.

The PE instruction stream hand-interleaves qkv/vpack/proj filler work into the
score/attnV stream so the PE stays dense (HAM-warm) while pacing the ScalarE
exp stream (the bottleneck). The last q-block's normalize + c_proj are split
into column halves so most of the tail overlaps the final exps.
"""

import sys

if "/opt/trn_rl_repo" not in sys.path:
    sys.path.insert(0, "/opt/trn_rl_repo")

import numpy as np
import ml_dtypes

BF16 = ml_dtypes.bfloat16

B, S, D = 2, 2048, 1024
H, hd, C = 16, 64, 32
NCORES = 8
HPC = 4            # heads per core
SB = 512           # q block
NSB = S // SB      # 4
KT = 128           # keys per tile
NKT = S // KT      # 16
DC = D // 128      # 8 contraction chunks
MB = 3             # qkv m-blocks: q', k_c, v_c

_cache = {}


def _build():
    import concourse.bacc as bacc
    import concourse.tile as tile
    import concourse.mybir as mybir

    dt = mybir.dt
    f32, bf16 = dt.float32, dt.bfloat16
    Exp = mybir.ActivationFunctionType.Exp
    mult = mybir.AluOpType.mult

    nc = bacc.Bacc("TRN2", target_bir_lowering=False, debug=False, num_devices=NCORES)

    hidden_t = nc.dram_tensor("hidden_t", [D, S], bf16, kind="ExternalInput")
    w_qkv = nc.dram_tensor("w_qkv", [D, MB * 128], bf16, kind="ExternalInput")
    b_qkv = nc.dram_tensor("b_qkv", [128, MB], f32, kind="ExternalInput")
    w_projp = nc.dram_tensor("w_projp", [128, D], bf16, kind="ExternalInput")
    mask_in = nc.dram_tensor("mask_in", [128, 4 * KT], bf16, kind="ExternalInput")
    ident_in = nc.dram_tensor("ident_in", [128, 128], bf16, kind="ExternalInput")
    out_t = nc.dram_tensor("out_t", [D, S], bf16, kind="ExternalOutput")

    with tile.TileContext(nc) as tc:
        with (
            tc.tile_pool(name="persist", bufs=1) as pp,
            tc.tile_pool(name="epool", bufs=6) as ep,
            tc.tile_pool(name="npool", bufs=2) as npo,
            tc.tile_pool(name="ostage", bufs=6) as op,
            tc.tile_pool(name="dscr", bufs=2, space="DRAM") as dr,
            tc.tile_pool(name="ps_scA", bufs=1, space="PSUM") as ps_scA,
            tc.tile_pool(name="ps_scB", bufs=1, space="PSUM") as ps_scB,
            tc.tile_pool(name="ps_at", bufs=1, space="PSUM") as ps_at,
            tc.tile_pool(name="ps_dn", bufs=1, space="PSUM") as ps_dn,
            tc.tile_pool(name="ps_big", bufs=2, space="PSUM") as ps_big,
        ):
            # ---- weights + hidden, few big DMAs, in consumption order ----
            wq_all = pp.tile([128, DC * MB * 128], bf16, tag="wq", name="wq_all")
            wqv = wq_all[:].rearrange("p (d c) -> p d c", d=DC)
            nc.sync.dma_start(
                wqv[:, :, 0:128],
                w_qkv.ap()[:, 0:128].rearrange("(d p) c -> p d c", d=DC),
            )
            bias = pp.tile([128, MB], f32, tag="bias", name="bias")
            nc.sync.dma_start(bias[:], b_qkv.ap())
            hT_all = pp.tile([128, DC * S], bf16, tag="hT", name="hT_all")
            hv = hT_all[:].rearrange("p (d s) -> p d s", d=DC)
            for q in range(2):
                nc.sync.dma_start(
                    hv[:, 4 * q:4 * q + 4, 0:SB],
                    hidden_t.ap()[512 * q:512 * q + 512, 0:SB]
                    .rearrange("(d p) s -> p d s", d=4),
                )
            nc.sync.dma_start(
                wqv[:, :, 128:256],
                w_qkv.ap()[:, 128:256].rearrange("(d p) c -> p d c", d=DC),
            )
            nc.sync.dma_start(
                wqv[:, :, 256:384],
                w_qkv.ap()[:, 256:384].rearrange("(d p) c -> p d c", d=DC),
            )
            ident = pp.tile([128, 128], bf16, tag="ident", name="ident")
            nc.sync.dma_start(ident[:], ident_in.ap())
            mask4 = pp.tile([128, 4 * KT], bf16, tag="mask4", name="mask4")
            nc.sync.dma_start(mask4[:], mask_in.ap())
            nc.sync.dma_start(
                hv[:, :, SB:S],
                hidden_t.ap()[:, SB:S].rearrange("(d p) s -> p d s", d=DC),
            )
            wpj = pp.tile([128, D], bf16, tag="wpj", name="wpj")
            nc.sync.dma_start(wpj[:], w_projp.ap())

            def wq_sl(d, mb):
                return wq_all[:, d * (MB * 128) + mb * 128:d * (MB * 128) + (mb + 1) * 128]

            def hT_sl(d, lo, hi):
                return hT_all[:, d * S + lo:d * S + hi]

            ones = pp.tile([128, 1], bf16, tag="ones", name="ones")
            nc.vector.memset(ones[:], 1.0)
            # preload the Exp table set early (off the critical path)
            warm = pp.tile([128, 1], bf16, tag="warm", name="warm")
            nc.scalar.activation(warm[:], ones[:], Exp)

            # qkv'^T destinations
            qp = pp.tile([128, S], bf16, tag="qp", name="qp")
            kcT = pp.tile([128, S], bf16, tag="kcT", name="kcT")
            vcT = pp.tile([128, S], bf16, tag="vcT", name="vcT")
            dests = [qp, kcT, vcT]
            vpack = [pp.tile([128, 128], bf16, tag=f"vpk{t}", name=f"vpk{t}")
                     for t in range(NKT)]

            def qkv_group(sb, mb, lo=0, hi=SB):
                ps = ps_big.tile([128, SB], f32, tag="big", name="psQ")
                w = hi - lo
                for d in range(DC):
                    nc.tensor.matmul(
                        ps[:, 0:w],
                        wq_sl(d, mb),
                        hT_sl(d, sb * SB + lo, sb * SB + hi),
                        start=(d == 0),
                        stop=(d == DC - 1),
                    )
                nc.vector.tensor_scalar_add(
                    out=dests[mb][:, sb * SB + lo:sb * SB + hi],
                    in0=ps[:, 0:w],
                    scalar1=bias[:, mb:mb + 1],
                )

            def vc_tr(kt):
                # vpack[kt][key, 32h+c] = vcT[32h+c, kt*128+key] via PE transpose
                psT = ps_big.tile([128, 128], bf16, tag="big", name="psT")
                nc.tensor.transpose(
                    psT[:], vcT[:, kt * KT:(kt + 1) * KT], ident[:]
                )
                nc.vector.tensor_copy(vpack[kt][:], psT[:])

            es = {}

            def scores_half(qsb, kb, half):
                # half 0: heads 0,1 -> scA; half 1: heads 2,3 -> scB
                r = kb - 4 * qsb
                c0 = max(r, 0) * KT
                pool = ps_scA if half == 0 else ps_scB
                sc = pool.tile([128, 2 * SB], f32, tag=f"sc{half}", name="sc")
                for hh in range(2):
                    h = 2 * half + hh
                    nc.tensor.matmul(
                        sc[:, hh * SB + c0:(hh + 1) * SB],
                        kcT[32 * h:32 * h + 32, kb * KT:(kb + 1) * KT],
                        qp[32 * h:32 * h + 32, qsb * SB + c0:(qsb + 1) * SB],
                        tile_position=(32 * h, 0),
                    )
                if half == 0:
                    es[(qsb, kb)] = ep.tile([128, 4 * SB], bf16, tag="e", name="e")
                e = es[(qsb, kb)]
                scv = sc[:].rearrange("p (h w) -> p h w", h=2)[:, :, c0:SB]
                ev = e[:].rearrange("p (h w) -> p h w", h=4)[:, 2 * half:2 * half + 2, c0:SB]
                nc.scalar.activation(ev, scv, Exp)
                if r >= 0 and half == 1:
                    # mask the diagonal 128-col block of each head (all 4)
                    ed = e[:].rearrange("p (h w) -> p h w", h=4)[:, :, c0:c0 + KT]
                    mv = mask4[:].rearrange("p (h w) -> p h w", h=4)
                    nc.vector.tensor_tensor(ed, ed, mv, mult)

            def scores_group(qsb, kb):
                scores_half(qsb, kb, 0)
                scores_half(qsb, kb, 1)

            attn_ps = {}
            den_ps = {}

            def attn_group(qsb, kb):
                r = kb - 4 * qsb
                c0 = max(r, 0) * KT
                nkb = 4 * qsb + 4
                if kb == 0:
                    attn_ps[qsb] = ps_at.tile([128, SB], f32, tag="at", name="at")
                    den_ps[qsb] = ps_dn.tile([128, SB], f32, tag="dn", name="dn")
                at, dn = attn_ps[qsb], den_ps[qsb]
                e = es.pop((qsb, kb))
                for h in range(HPC):
                    ee = e[:, h * SB + c0:(h + 1) * SB]
                    nc.tensor.matmul(
                        at[32 * h:32 * h + 32, c0:SB], vpack[kb][:, 32 * h:32 * h + 32],
                        ee, tile_position=(0, 32 * h),
                        start=(kb == 0), stop=(kb == nkb - 1),
                    )
                    nc.tensor.matmul(
                        dn[32 * h:32 * h + 1, c0:SB], ones[:, 0:1],
                        ee, tile_position=(0, 32 * h),
                        start=(kb == 0), stop=(kb == nkb - 1),
                    )

            attn_norm = {}

            def norm(qsb, lo, hi, key, eng=None):
                # normalize attn columns [lo, hi) of q-block qsb
                w = hi - lo
                eng = eng or nc.sync
                at, dn = attn_ps[qsb], den_ps[qsb]
                attn_sb = npo.tile([128, w], bf16, tag=f"attn_sb{w}", name="attn_sb")
                nc.vector.tensor_copy(attn_sb[:], at[:, lo:hi])
                den_sb = npo.tile([128, w], bf16, tag=f"den_sb{w}", name="den_sb")
                nc.vector.tensor_copy(den_sb[:], dn[:, lo:hi])
                # bounce den rows {32h} -> [128, w/32] for a cheap DVE reciprocal
                den_dr = dr.tile([HPC, w], bf16, tag=f"den_dr{w}", name="den_dr")
                eng.dma_start(
                    den_dr[:], den_sb[:].rearrange("(h s) q -> h s q", s=32)[:, 0, :]
                )
                j = w // 32
                den_c = npo.tile([128, j], bf16, tag=f"den_c{w}", name="den_c")
                eng.dma_start(
                    den_c[:], den_dr[:].rearrange("h (p j) -> (h p) j", j=j)
                )
                rec_c = npo.tile([128, j], bf16, tag=f"rec_c{w}", name="rec_c")
                with nc.allow_low_precision(reason="softmax denom recip in bf16"):
                    nc.vector.reciprocal(rec_c[:], den_c[:])
                rec_dr = dr.tile([HPC, w], bf16, tag=f"rec_dr{w}", name="rec_dr")
                eng.dma_start(
                    rec_dr[:].rearrange("h (p j) -> (h p) j", j=j), rec_c[:]
                )
                recb = npo.tile([128, w], bf16, tag=f"recb{w}", name="recb")
                for h in range(HPC):
                    eng.dma_start(
                        recb[32 * h:32 * h + 32, :],
                        rec_dr[h:h + 1, :].to_broadcast([32, w]),
                    )
                an = npo.tile([128, w], bf16, tag=f"an{w}", name="an")
                nc.vector.tensor_tensor(an[:], attn_sb[:], recb[:], mult)
                attn_norm[key] = (an, lo, hi)

            def proj_group(key, mb, tail=False):
                an, lo, hi = attn_norm[key]
                qsb = key if isinstance(key, int) else key[0]
                w = hi - lo
                ps = ps_big.tile([128, SB], f32, tag="big", name="psP")
                nc.tensor.matmul(ps[:, 0:w], wpj[:, mb * 128:(mb + 1) * 128], an[:])
                stage = op.tile([128, w], bf16, tag=f"stage{w}", name="stage")
                # in the tail the exps are done: route evac+DMA through the
                # idle Scalar engine for every other block to halve the chain
                if tail and mb % 2 == 1:
                    nc.scalar.copy(stage[:], ps[:, 0:w])
                    dma_eng = nc.scalar
                else:
                    nc.vector.tensor_copy(stage[:], ps[:, 0:w])
                    dma_eng = nc.sync
                dma_eng.dma_start(
                    out_t.ap()[mb * 128:(mb + 1) * 128,
                               qsb * SB + lo:qsb * SB + hi],
                    stage[:],
                )

            # ================= PE schedule =================
            def QKV(sb, mb):
                return lambda: qkv_group(sb, mb)

            def VCT(kt):
                return lambda: vc_tr(kt)

            def PRJ(key, mb):
                return lambda: proj_group(key, mb)

            def NRM3A():
                def f():
                    norm(3, 0, 256, (3, 0))
                    for mb in range(8):
                        proj_group((3, 0), mb, tail=True)
                return f

            def NRM(qsb):
                def f():
                    norm(qsb, 0, SB, qsb)
                    attn_ps.pop(qsb)
                    den_ps.pop(qsb)
                return f

            # flat stream of all 40 (qsb, kb) score groups; the attnV stream
            # lags one group behind globally (including across qsb boundaries,
            # so the exp stream never stalls at a boundary); fillers per slot
            groups = [(q, k) for q in range(NSB) for k in range(4 * q + 4)]
            fillers = {
                1: [QKV(1, 0)], 2: [VCT(2)], 3: [VCT(3)],
                5: [QKV(1, 1)], 6: [QKV(1, 2)], 7: [QKV(2, 0)],
                8: [VCT(4)], 9: [VCT(5)], 10: [VCT(6)], 11: [VCT(7)],
                13: [QKV(2, 1)], 14: [QKV(2, 2)], 15: [QKV(3, 0)],
                16: [PRJ(0, 0)], 17: [PRJ(0, 1), VCT(8)],
                18: [PRJ(0, 2), VCT(9)], 19: [PRJ(0, 3), VCT(10)],
                20: [PRJ(0, 4), VCT(11)], 21: [PRJ(0, 5)], 22: [PRJ(0, 6)],
                23: [PRJ(0, 7)],
                25: [QKV(3, 1)], 26: [QKV(3, 2)], 27: [PRJ(1, 0)],
                28: [PRJ(1, 1)], 29: [PRJ(1, 2)], 30: [PRJ(1, 3)],
                31: [PRJ(1, 4)], 32: [PRJ(1, 5)],
                33: [PRJ(1, 6), VCT(12)], 34: [PRJ(1, 7), VCT(13)],
                35: [PRJ(2, 0), VCT(14)], 36: [PRJ(2, 1), VCT(15)],
                37: [PRJ(2, 2), PRJ(2, 3)], 38: [PRJ(2, 4), PRJ(2, 5)],
                39: [PRJ(2, 6), PRJ(2, 7)],
            }

            # lead-in: q' of sb0, then k_c keys 0:256 only, so the first
            # scores+exp fire before the rest of sb0's projections
            qkv_group(0, 0)
            qkv_group(0, 1, 0, 256)
            scores_group(0, 0)
            scores_group(0, 1)
            qkv_group(0, 1, 256, SB)
            qkv_group(0, 2)
            vc_tr(0)
            vc_tr(1)

            for i, (qsb, kb) in enumerate(groups):
                if i == 38:
                    # qsb3 col 0:256 of attn/den is final after A(3,13):
                    # kick its normalize early so proj3a overlaps the tail
                    attn_group(3, 13)
                    norm(3, 0, 256, (3, 0))
                if i > 1:
                    scores_group(qsb, kb)
                pq, pk = groups[i - 1] if i > 0 else (None, None)
                if i > 0 and not (pq == 3 and pk == 13):
                    attn_group(pq, pk)
                    if pq < qsb:
                        norm(pq, 0, SB, pq)
                        attn_ps.pop(pq)
                        den_ps.pop(pq)
                for thunk in fillers.get(i, []):
                    thunk()
            attn_group(3, 15)
            # tail: second half of qsb3 (norm3b DMAs ride the idle ACT queue)
            norm(3, 256, SB, (3, 1), eng=nc.scalar)
            attn_ps.pop(3)
            den_ps.pop(3)
            for mb in range(8):
                proj_group((3, 0), mb, tail=True)
            for mb in range(8):
                proj_group((3, 1), mb, tail=True)

    nc.compile()
    return nc


def _prep_inputs(hidden_states, w_attn, b_attn, wk_c, wv_c, wk_d, wv_d, w_proj):
    """Per-core input maps (host-side shard + fold + pack + bf16 cast)."""
    f8 = np.float64
    hidden_T = [np.ascontiguousarray(hidden_states[b].T).astype(BF16) for b in range(B)]
    w_attn8, b_attn8 = w_attn.astype(f8), b_attn.astype(f8)
    scl = 1.0 / np.sqrt(hd)

    k = np.arange(128).reshape(128, 1)
    j = np.arange(KT).reshape(1, KT)
    mask1 = (k <= j).astype(BF16)
    mask4 = np.tile(mask1, (1, 4))
    ident = np.eye(128, dtype=BF16)

    in_maps = []
    for c in range(NCORES):
        b = c // 4
        hs = [4 * (c % 4) + h for h in range(HPC)]
        wq_cols, bq = [], []
        wk_cols, bk = [], []
        wv_cols, bv = [], []
        for h in hs:
            wq_cols.append(w_attn8[:, h * hd:(h + 1) * hd] @ wk_d[h].astype(f8).T * scl)
            bq.append(b_attn8[h * hd:(h + 1) * hd] @ wk_d[h].astype(f8).T * scl)
            wk_cols.append(w_attn8[:, D + h * hd:D + (h + 1) * hd] @ wk_c[h].astype(f8))
            bk.append(b_attn8[D + h * hd:D + (h + 1) * hd] @ wk_c[h].astype(f8))
            wv_cols.append(
                w_attn8[:, 2 * D + h * hd:2 * D + (h + 1) * hd] @ wv_c[h].astype(f8))
            bv.append(b_attn8[2 * D + h * hd:2 * D + (h + 1) * hd] @ wv_c[h].astype(f8))
        w_qkv_l = np.concatenate(
            [np.concatenate(wq_cols, 1), np.concatenate(wk_cols, 1),
             np.concatenate(wv_cols, 1)], axis=1,
        ).astype(BF16)                                          # [1024, 384]
        b_qkv_l = np.stack(
            [np.concatenate(bq), np.concatenate(bk), np.concatenate(bv)],
            axis=1,
        ).astype(np.float32)                                    # [128, 3]
        w_projp_l = np.concatenate(
            [wv_d[h].astype(f8) @ w_proj[h * hd:(h + 1) * hd, :].astype(f8)
             for h in hs], axis=0,
        ).astype(BF16)                                          # [128, 1024]
        in_maps.append(
            {
                "hidden_t": hidden_T[b],
                "w_qkv": w_qkv_l,
                "b_qkv": b_qkv_l,
                "w_projp": w_projp_l,
                "mask_in": mask4,
                "ident_in": ident,
            }
        )
    return in_maps


def kernel(
    hidden_states,
    w_attn,
    b_attn,
    w_proj,
    b_proj,
    wk_c,
    wv_c,
    wk_d,
    wv_d,
    _trace=False,
):
    from concourse.bass_utils import run_bass_kernel_spmd

    if "nc" not in _cache:
        _cache["nc"] = _build()
    nc = _cache["nc"]

    in_maps = _prep_inputs(
        np.asarray(hidden_states),
        np.asarray(w_attn),
        np.asarray(b_attn),
        np.asarray(wk_c),
        np.asarray(wv_c),
        np.asarray(wk_d),
        np.asarray(wv_d),
        np.asarray(w_proj),
    )
    res = run_bass_kernel_spmd(
        nc, in_maps, core_ids=list(range(NCORES)), trace=_trace
    )
    out = np.empty((B, S, D), np.float32)
    for b in range(B):
        acc = np.zeros((D, S), np.float32)
        for c in range(4 * b, 4 * b + 4):
            acc += res.results[c]["out_t"].astype(np.float32)
        out[b] = acc.T + np.asarray(b_proj, np.float32)
    if _trace:
        _cache["last_exec_time_ns"] = res.exec_time_ns
        _cache["last_results"] = res
    return out
